# revision 37
# baseline (speedup 1.0000x reference)
"""DeepSeek-V2 MLA decoder layer (prefill, T=2048) on 8 Trainium2 NeuronCores.

v3: all big GEMMs (qkv_a, q_b, kv_b, scores, o_proj) run as 3-term
compensated e4m3 DoubleRow matmuls: X ~= Xhi+Xlo, W ~= Whi+Wlo (host- or
device-split, power-of-2 scaled), out = M1 + M2 where
  M1 = sum_k Xhi[k] Whi[k]      (DR over k-tile pairs, 0.25x f16 cost)
  M2 = sum_k (Xhi[k] Wlo[k] + Xlo[k] Whi[k])   (one DR per k-tile, 0.5x)
PV stays f16.  Collectives carry hi+lo fp8 pairs (same bytes as f16).
Overall structure as v2: token-parallel stage 1 with early kv-latent
AllGather and per-head-pair q AllToAll; head-parallel causal attention
in the S^T formulation; partial o_proj per core, host sums.
"""
import numpy as np
import ml_dtypes

import concourse.bass as bass
import concourse.mybir as mybir
import concourse.tile as tile
from concourse import bacc
from concourse.bass_utils import run_bass_kernel_spmd
from concourse.masks import make_identity

F16 = mybir.dt.float16
F32 = mybir.dt.float32
F8 = mybir.dt.float8e4
E4 = ml_dtypes.float8_e4m3
AX = mybir.AxisListType
AF = mybir.ActivationFunctionType
DRM = mybir.MatmulPerfMode.DoubleRow

NCORES = 8
T, HID, H = 2048, 5120, 32
DN, DR, DV, QL, KL = 128, 64, 128, 1536, 512
EPS = 1e-6
THETA = 10000.0
HPC = H // NCORES            # 4 heads per core
TPC = T // NCORES            # 256 tokens per core
CW = QL + KL + DR            # 2112
SM_SCALE = float((DN + DR) ** -0.5)
EXP_BIAS = float(-7.0 * np.log(2.0))
NEG = -1e9
QTILES = T // 128            # 16

# fixed power-of-2 scales for device-quantized activations
SQA = 16.0    # q_a latent
SKV = 16.0    # kv latent
SA = 16.0     # q/k score operands
SAT = 16.0    # attn output

_PROGRAM_CACHE = {}


def _pow2(target, amax):
    return float(2.0 ** np.floor(np.log2(target / max(amax, 1e-30))))


def _hilo(x):
    hi = x.astype(E4)
    lo = (x - hi.astype(np.float32)).astype(E4)
    return hi, lo


def sap(t, off, dims, p=128):
    fa = t[:]
    return bass.AP(tensor=fa.tensor, offset=fa.offset + off,
                   ap=[[fa.ap[0][0], p]] + [list(d) for d in dims])


def build_program(scales):
    key = tuple(sorted(scales.items()))
    if key in _PROGRAM_CACHE:
        return _PROGRAM_CACHE[key]
    c1 = scales["sh"] * scales["swa"]          # qkv_a psum scale
    cqn = SA / (SQA * scales["swqn"])          # q nope psum -> wire scale
    ckt = SA / (SKV * scales["swkvb"])         # k psum -> score scale
    cv = 1.0 / (SKV * scales["swkvb"])         # v psum -> true scale
    co = 1.0 / (SAT * scales["swo"])           # o_proj psum -> true scale
    inv_c1 = 1.0 / c1

    nc = bacc.Bacc("TRN2", target_bir_lowering=False, debug=False,
                   num_devices=NCORES)

    hT8_d = nc.dram_tensor("hT8", [128, 40, 2, TPC], F8,
                           kind="ExternalInput").ap()
    wa8_d = nc.dram_tensor("wa8", [128, 40, 2, CW - DR], F8,
                           kind="ExternalInput").ap()
    wape8_d = nc.dram_tensor("wape8", [128, 40, 2, DR], F8,
                             kind="ExternalInput").ap()
    wqbn8_d = nc.dram_tensor("wqbn8", [128, 12, 2, H * DN], F8,
                             kind="ExternalInput").ap()
    wqbp8_d = nc.dram_tensor("wqbp8", [128, 12, 8, 2, 256], F8,
                             kind="ExternalInput").ap()
    wkvb8_d = nc.dram_tensor("wkvb8", [128, 4, 2, HPC * 256], F8,
                             kind="ExternalInput").ap()
    wo8_d = nc.dram_tensor("wo8", [128, HPC, 2, HID], F8,
                           kind="ExternalInput").ap()
    ctok_d = nc.dram_tensor("ctok", [TPC, 32], F16, kind="ExternalInput").ap()
    stok_d = nc.dram_tensor("stok", [TPC, 32], F16, kind="ExternalInput").ap()
    cosr_d = nc.dram_tensor("cosr", [TPC, 512], F16, kind="ExternalInput").ap()
    sinr_d = nc.dram_tensor("sinr", [TPC, 512], F16, kind="ExternalInput").ap()
    triT_d = nc.dram_tensor("triT", [128, 128], F32, kind="ExternalInput").ap()
    out_d = nc.dram_tensor("out", [T, HID], F16, kind="ExternalOutput").ap()

    with tile.TileContext(nc) as tc:
        with (
            tc.tile_pool(name="const", bufs=1) as cst,
            tc.tile_pool(name="dram", bufs=1, space="DRAM") as dram,
            tc.tile_pool(name="dram2", bufs=4, space="DRAM") as dram2,
            tc.tile_pool(name="proje", bufs=1) as proje,
        ):
            ident16 = cst.tile([128, 128], F16, tag="id16")
            make_identity(nc, ident16[:])
            ones16 = cst.tile([128, 1], F16, tag="ones16")
            nc.vector.memset(ones16[:], 1.0)
            triT_sb = cst.tile([128, 128], F32, tag="triT")
            nc.gpsimd.dma_start(triT_sb[:], triT_d[:])
            ctok_sb = cst.tile([128, 2, 32], F16, tag="ctok")
            nc.gpsimd.dma_start(ctok_sb[:], ctok_d.rearrange("(a p) f -> p a f", p=128))
            stok_sb = cst.tile([128, 2, 32], F16, tag="stok")
            nc.gpsimd.dma_start(stok_sb[:], stok_d.rearrange("(a p) f -> p a f", p=128))
            cosr_sb = cst.tile([128, 2, 512], F16, tag="cosr")
            nc.gpsimd.dma_start(cosr_sb[:], cosr_d.rearrange("(a p) f -> p a f", p=128))
            sinr_sb = cst.tile([128, 2, 512], F16, tag="sinr")
            nc.gpsimd.dma_start(sinr_sb[:], sinr_d.rearrange("(a p) f -> p a f", p=128))
            epss_sb = cst.tile([128, 1], F32, tag="epss")
            nc.vector.memset(epss_sb[:], float(EPS / (SQA * SQA)))
            ebias_sb = cst.tile([128, 1], F32, tag="ebias")
            nc.vector.memset(ebias_sb[:], EXP_BIAS)
            warm = cst.tile([128, 1], F32, tag="warm")
            nc.vector.memset(warm[:], 1.0)
            wsink = cst.tile([128, 4], F32, tag="wsink")
            nc.scalar.activation(wsink[:, 0:1], warm[:], AF.Square)
            nc.scalar.activation(wsink[:, 1:2], warm[:], AF.Sqrt)
            nc.scalar.activation(wsink[:, 2:3], warm[:], AF.Exp)
            nc.scalar.activation(wsink[:, 3:4], warm[:], AF.Copy)

            ag2_in = dram.tile([2 * (KL + DR), TPC], F8, tag="ag2in")
            ag2_out = dram.tile([NCORES * 2 * (KL + DR), TPC], F8,
                                addr_space="Shared", tag="ag2out")
            a2a_in = [dram.tile([NCORES * 384, TPC], F8, tag=f"a2ain{p}",
                                name=f"a2ain{p}") for p in range(2)]
            a2a_out = [dram.tile([NCORES * 384, TPC], F8, tag=f"a2aout{p}",
                                 name=f"a2aout{p}") for p in range(2)]

            # ---------------- Stage 1
            with (
                tc.tile_pool(name="ph1", bufs=1) as ph1,
                tc.tile_pool(name="ph1w", bufs=4) as ph1w,
                tc.tile_pool(name="ph1pe", bufs=1) as ph1pe,
                tc.tile_pool(name="ph1qw", bufs=3) as ph1qw,
                tc.tile_pool(name="ph1s", bufs=4) as ph1s,
                tc.tile_pool(name="ph1r", bufs=1) as ph1r,
                tc.tile_pool(name="ph1n", bufs=3) as ph1n,
                tc.tile_pool(name="ph1ps", bufs=2, space="PSUM") as ph1ps,
                tc.tile_pool(name="ph1tp", bufs=2, space="PSUM") as ph1tp,
                tc.tile_pool(name="ph1qps", bufs=2, space="PSUM") as ph1qps,
            ):
                stage = [ph1.tile([128, CW], F16, tag=f"stage{tt}",
                                  name=f"stage{tt}") for tt in range(2)]
                hT8_sb = ph1.tile([128, 40, 2, TPC], F8, tag="hT8")
                for kg in range(4):
                    nc.scalar.dma_start(
                        hT8_sb[:, kg * 10:(kg + 1) * 10, :, :],
                        hT8_d[:, kg * 10:(kg + 1) * 10, :, :])

                # x3 DR matmul emission for qkv_a.  hT8_sb layout
                # [p, j(40), t(256), hilo(2)]; wa_t [p, jl(8), (lo,hi), w].
                def qkv_x3(n0, w, kv=False):
                    ps = [ph1ps.tile([128, w], F32, tag=f"s1ps{tt}",
                                     name=f"s1ps{tt}") for tt in range(2)]
                    for kg in range(5):
                        wa_t = ph1w.tile([128, 8, 2, w], F8, tag="wa_t",
                                         name="wa_t")
                        nc.sync.dma_start(
                            wa_t[:], wa8_d[:, kg * 8:(kg + 1) * 8, :,
                                           n0:n0 + w])
                        for tt in range(2):
                            for jp in range(4):
                                j = kg * 8 + 2 * jp
                                lhs = sap(hT8_sb, j * 512 + tt * 128,
                                          [[512, 2], [1, 128]])
                                rhs = sap(wa_t, (2 * jp) * 2 * w + w,
                                          [[2 * w, 2], [1, w]])
                                nc.tensor.matmul(
                                    ps[tt][:], lhs, rhs,
                                    start=(kg == 0 and jp == 0), stop=False,
                                    perf_mode=DRM)
                            for jl in range(8):
                                j = kg * 8 + jl
                                lhs = sap(hT8_sb, j * 512 + tt * 128,
                                          [[256, 2], [1, 128]])
                                rhs = sap(wa_t, jl * 2 * w,
                                          [[w, 2], [1, w]])
                                nc.tensor.matmul(
                                    ps[tt][:], lhs, rhs, start=False,
                                    stop=(kg == 4 and jl == 7),
                                    perf_mode=DRM)
                    return ps

                # pe slice: all 40 k-tiles in one weight load
                def qkv_x3_pe():
                    w = DR
                    ps = [ph1ps.tile([128, w], F32, tag=f"s1ps{tt}",
                                     name=f"s1ps{tt}") for tt in range(2)]
                    wa_t = ph1pe.tile([128, 40, 2, w], F8, tag="wa_pe")
                    nc.sync.dma_start(wa_t[:], wape8_d[:])
                    for tt in range(2):
                        for jp in range(20):
                            lhs = sap(hT8_sb, (2 * jp) * 512 + tt * 128,
                                      [[512, 2], [1, 128]])
                            rhs = sap(wa_t, (2 * jp) * 2 * w + w,
                                      [[2 * w, 2], [1, w]])
                            nc.tensor.matmul(ps[tt][:], lhs, rhs,
                                             start=(jp == 0), stop=False,
                                             perf_mode=DRM)
                        for j in range(40):
                            lhs = sap(hT8_sb, j * 512 + tt * 128,
                                      [[256, 2], [1, 128]])
                            rhs = sap(wa_t, j * 2 * w, [[w, 2], [1, w]])
                            nc.tensor.matmul(ps[tt][:], lhs, rhs, start=False,
                                             stop=(j == 39), perf_mode=DRM)
                    return ps

                # wa col layout: [kv 512 | pe 64 | q 1536]
                kvps = qkv_x3(0, KL, kv=True)
                peps = qkv_x3_pe()

                for tt in range(2):
                    sums = ph1s.tile([128, 4], F32, tag="s1sums")
                    dump = ph1s.tile([128, 512], F16, tag="s1dump")
                    nc.scalar.activation(dump[:], kvps[tt][:], AF.Square,
                                         scale=inv_c1,
                                         accum_out=sums[:, 3:4])
                    rkv = ph1s.tile([128, 1], F32, tag="rkv")
                    nc.scalar.activation(rkv[:], sums[:, 3:4], AF.Sqrt,
                                         bias=epss_sb[:],
                                         scale=float(1.0 / (KL * SKV * SKV)))
                    nc.vector.reciprocal(rkv[:], rkv[:])
                    # rkv = SKV / rms(x); psum = c1*x -> scale by rkv*inv_c1
                    rkv2 = ph1s.tile([128, 1], F32, tag="rkv2")
                    nc.vector.tensor_scalar_mul(rkv2[:], rkv[:],
                                                float(inv_c1))
                    kva16 = ph1.tile([128, KL], F16, tag=f"kva16_{tt}",
                                     name=f"kva16_{tt}")
                    nc.scalar.activation(kva16[:], kvps[tt][:],
                                         AF.Copy, scale=rkv2[:])
                    kpe16 = ph1.tile([128, 64], F16, tag=f"kpe16_{tt}",
                                     name=f"kpe16_{tt}")
                    pe = peps[tt][:]
                    ct, st = ctok_sb[:, tt, :], stok_sb[:, tt, :]
                    t1 = ph1s.tile([128, 32], F32, tag="rt1")
                    t2 = ph1s.tile([128, 32], F32, tag="rt2")
                    nc.vector.tensor_mul(t1[:], pe[:, 0:32], ct)
                    nc.vector.tensor_mul(t2[:], pe[:, 32:64], st)
                    nc.vector.tensor_sub(kpe16[:, 0:32], t1[:], t2[:])
                    t3 = ph1s.tile([128, 32], F32, tag="rt3")
                    t4 = ph1s.tile([128, 32], F32, tag="rt4")
                    nc.vector.tensor_mul(t3[:], pe[:, 32:64], ct)
                    nc.vector.tensor_mul(t4[:], pe[:, 0:32], st)
                    nc.vector.tensor_add(kpe16[:, 32:64], t3[:], t4[:])

                    # transpose + hi/lo quantize -> ag2_in rows
                    # [kva_hi 0-511 | kva_lo 512-1023 | kpe_lo 1024-1087 |
                    #  kpe_hi 1088-1151]
                    for b in range(4):
                        tp = ph1tp.tile([128, 128], F16, tag="s1tp",
                                        name="s1tp")
                        nc.tensor.transpose(tp[:],
                                            kva16[:, b * 128:(b + 1) * 128],
                                            ident16[:])
                        hl8 = ph1s.tile([128, 2, 128], F8, tag="kvhl",
                                        name="kvhl")
                        nc.vector.tensor_copy(hl8[:, 0, :], tp[:])
                        nc.vector.tensor_sub(hl8[:, 1, :], tp[:],
                                             hl8[:, 0, :])
                        dst = bass.AP(
                            tensor=ag2_in.tensor,
                            offset=ag2_in.offset + b * 128 * TPC + tt * 128,
                            ap=[[TPC, 128], [KL * TPC, 2], [1, 128]])
                        nc.scalar.dma_start(dst, hl8[:])
                    tp2f = ph1tp.tile([128, 128], F16, tag="s1tp", name="s1tp")
                    nc.tensor.transpose(tp2f[0:64, :], kpe16[:], ident16[:])
                    phl = ph1s.tile([64, 2, 128], F8, tag="kphl")
                    nc.vector.tensor_copy(phl[:, 1, :], tp2f[0:64, :])
                    nc.vector.tensor_sub(phl[:, 0, :], tp2f[0:64, :],
                                         phl[:, 1, :])
                    dst = bass.AP(
                        tensor=ag2_in.tensor,
                        offset=ag2_in.offset + 2 * KL * TPC + tt * 128,
                        ap=[[TPC, 64], [64 * TPC, 2], [1, 128]])
                    nc.scalar.dma_start(dst, phl[:])

                nc.gpsimd.collective_compute(
                    "AllGather", mybir.AluOpType.bypass,
                    ins=[ag2_in.opt()], outs=[ag2_out.opt()],
                    replica_groups=[list(range(NCORES))])

                # copy q slices into stage (descaled to true values)
                def stage_copy(ps, n0, w):
                    for tt in range(2):
                        if tt == 0:
                            nc.scalar.activation(stage[tt][:, n0:n0 + w],
                                                 ps[tt][:], AF.Copy,
                                                 scale=float(inv_c1))
                        else:
                            nc.vector.tensor_scalar_mul(
                                stage[tt][:, n0:n0 + w], ps[tt][:],
                                float(inv_c1))

                qps0 = qkv_x3(KL, 512)
                stage_copy(qps0, KL + DR, 512)
                qps1 = qkv_x3(KL + 512, 512)
                stage_copy(qps1, KL + DR + 512, 512)
                qps2 = qkv_x3(KL + 1024, 512)
                stage_copy(qps2, KL + DR + 1024, 512)
                KVW = KL + DR
                qa16 = [None, None]
                for tt in range(2):
                    sums = ph1s.tile([128, 4], F32, tag="s1sums")
                    dump = ph1s.tile([128, 512], F16, tag="s1dump")
                    for i in range(3):
                        nc.scalar.activation(
                            dump[:], stage[tt][:, KVW + i * 512:KVW + (i + 1) * 512],
                            AF.Square, accum_out=sums[:, i:i + 1])
                    qs = ph1s.tile([128, 1], F32, tag="qs")
                    nc.vector.reduce_sum(qs[:], sums[:, 0:3], axis=AX.X)
                    rq = ph1s.tile([128, 1], F32, tag="rq")
                    nc.scalar.activation(rq[:], qs[:], AF.Sqrt,
                                         bias=epss_sb[:],
                                         scale=float(1.0 / (QL * SQA * SQA)))
                    nc.vector.reciprocal(rq[:], rq[:])   # = SQA / rms
                    qa16[tt] = ph1.tile([128, QL], F16, tag=f"qa16_{tt}",
                                        name=f"qa16_{tt}")
                    for i in range(3):
                        nc.scalar.activation(
                            qa16[tt][:, i * 512:(i + 1) * 512],
                            stage[tt][:, KVW + i * 512:KVW + (i + 1) * 512],
                            AF.Copy, scale=rq[:])

                # q_aT hi/lo: [128, c(12), (hi,lo), 256] via PE transposes
                qaT8 = ph1.tile([128, 12, 2, TPC], F8, tag="qaT8")
                for tt in range(2):
                    for c in range(12):
                        tp = ph1tp.tile([128, 128], F16, tag="s1tp",
                                        name="s1tp")
                        nc.tensor.transpose(tp[:],
                                            qa16[tt][:, c * 128:(c + 1) * 128],
                                            ident16[:])
                        nc.vector.tensor_copy(
                            qaT8[:, c, 0, tt * 128:(tt + 1) * 128], tp[:])
                        nc.vector.tensor_sub(
                            qaT8[:, c, 1, tt * 128:(tt + 1) * 128], tp[:],
                            qaT8[:, c, 0, tt * 128:(tt + 1) * 128])

                # q_b x3 helpers: stationary = weights [p, c, (lo,hi), cols]
                def qb_mm(pq, wq8, wcols, col0, ncol):
                    for cp in range(6):
                        lhs = sap(wq8, (2 * cp) * 2 * wcols + wcols + col0,
                                  [[2 * wcols, 2], [1, ncol]])
                        rhs = sap(qaT8, (2 * cp) * 2 * TPC,
                                  [[2 * TPC, 2], [1, TPC]])
                        nc.tensor.matmul(pq[:], lhs, rhs, start=(cp == 0),
                                         stop=False, perf_mode=DRM)
                    for c in range(12):
                        lhs = sap(wq8, c * 2 * wcols + col0,
                                  [[wcols, 2], [1, ncol]])
                        rhs = sap(qaT8, c * 2 * TPC, [[TPC, 2], [1, TPC]])
                        nc.tensor.matmul(pq[:], lhs, rhs, start=False,
                                         stop=(c == 11), perf_mode=DRM)

                # wqbn col = pair*2048 + d*256 + (h%2)*128 + dn
                # wqbp col = pair*1024 + half*512 + d*64 + (h%2)*32 + f
                # a2a rows per dest: [hE_hi 128 | hO_hi 128 | peE_hi 64 |
                #   peO_hi 64 | hE_lo 128 | hO_lo 128 | peE_lo 64 | peO_lo 64]
                for pair in range(2):
                    qpe = ph1r.tile([128, 2, 1024], F32, tag=f"qpe{pair}",
                                    name=f"qpe{pair}")
                    # pe: token-stationary, out [128 tok, 256 pe cols]
                    for sg8 in range(4):
                        wp8 = ph1qw.tile([128, 12, 2, 256], F8, tag="wqp",
                                         name="wqp")
                        nc.sync.dma_start(
                            wp8[:], wqbp8_d[:, :, pair * 4 + sg8, :, :])
                        for tt in range(2):
                            pp = ph1qps.tile([128, 256], F32, tag="pq",
                                             name="pq")
                            for cp in range(6):
                                lhs = sap(qaT8, (2 * cp) * 512 + tt * 128,
                                          [[512, 2], [1, 128]])
                                rhs = sap(wp8, (2 * cp) * 512 + 256,
                                          [[512, 2], [1, 256]])
                                nc.tensor.matmul(pp[:], lhs, rhs,
                                                 start=(cp == 0), stop=False,
                                                 perf_mode=DRM)
                            for c in range(12):
                                lhs = sap(qaT8, c * 512 + tt * 128,
                                          [[256, 2], [1, 128]])
                                rhs = sap(wp8, c * 512,
                                          [[256, 2], [1, 256]])
                                nc.tensor.matmul(pp[:], lhs, rhs, start=False,
                                                 stop=(c == 11),
                                                 perf_mode=DRM)
                            nc.scalar.copy(
                                qpe[:, tt, sg8 * 256:(sg8 + 1) * 256], pp[:])

                    def emit_nope(mg):
                        nsb8 = ph1n.tile([128, 4, TPC], F8, tag="nsb",
                                         name="nsb")
                        wq8 = ph1qw.tile([128, 12, 2, 512], F8, tag="wqn",
                                         name="wqn")
                        nc.sync.dma_start(
                            wq8[:], wqbn8_d[:, :, :, (pair * 4 + mg) * 512:
                                            (pair * 4 + mg + 1) * 512])
                        for ml in range(4):
                            pq = ph1qps.tile([128, TPC], F32, tag="pq",
                                             name="pq")
                            qb_mm(pq, wq8, 512, ml * 128, 128)
                            nsb16 = ph1s.tile([128, TPC], F16, tag="nsb16",
                                              name="nsb16")
                            nc.scalar.activation(nsb16[:], pq[:], AF.Copy,
                                                 scale=float(cqn))
                            nc.vector.tensor_copy(nsb8[:, ml, :], nsb16[:])
                        for dl in range(2):
                            d = mg * 2 + dl
                            dst = bass.AP(
                                tensor=a2a_in[pair].tensor,
                                offset=a2a_in[pair].offset + d * 384 * TPC,
                                ap=[[TPC, 128], [128 * TPC, 2], [1, TPC]])
                            nc.scalar.dma_start(
                                dst, nsb8[:, 2 * dl:2 * dl + 2, :])

                    emit_nope(0)
                    for tt in range(2):
                        cr, sr = cosr_sb[:, tt, :], sinr_sb[:, tt, :]
                        qpe16 = ph1r.tile([128, 1024], F16, tag="qpe16",
                                          name="qpe16")
                        eE = ph1r.tile([128, 512], F32, tag="ropeE",
                                       name="ropeE")
                        eO = ph1r.tile([128, 512], F32, tag="ropeO",
                                       name="ropeO")
                        t2 = ph1r.tile([128, 512], F32, tag="ropet2",
                                       name="ropet2")
                        qq = qpe[:, tt, :]
                        nc.vector.tensor_mul(eE[:], qq[:, 0:512], cr)
                        nc.vector.tensor_mul(t2[:], qq[:, 512:1024], sr)
                        nc.vector.tensor_sub(qpe16[:, 0:512], eE[:], t2[:])
                        nc.vector.tensor_mul(eO[:], qq[:, 512:1024], cr)
                        nc.vector.tensor_mul(t2[:], qq[:, 0:512], sr)
                        nc.vector.tensor_add(qpe16[:, 512:1024], eO[:], t2[:])
                        # transpose per (half, d-pair); hi/lo -> pestg8
                        pestg8 = ph1n.tile([64, 2, 8, 128], F8,
                                           tag="pestg", name="pestg")
                        for half in range(2):
                            for d in range(0, 8, 2):
                                s0 = half * 512 + d * 64
                                tp = ph1tp.tile([128, 128], F16, tag="s1tp",
                                                name="s1tp")
                                nc.tensor.transpose(tp[:],
                                                    qpe16[:, s0:s0 + 128],
                                                    ident16[:])
                                nc.vector.tensor_copy(
                                    pestg8[:, half, d, :], tp[0:64, :])
                                nc.vector.tensor_copy(
                                    pestg8[:, half, d + 1, :],
                                    tp[64:128, :])
                        for half in range(2):
                            dst = bass.AP(
                                tensor=a2a_in[pair].tensor,
                                offset=a2a_in[pair].offset
                                + (256 + half * 64) * TPC + tt * 128,
                                ap=[[TPC, 64], [384 * TPC, 8], [1, 128]])
                            nc.scalar.dma_start(dst, pestg8[:, half, :, :])

                    for mg_i in range(1, 4):
                        emit_nope(mg_i)
                    nc.gpsimd.collective_compute(
                        "AllToAll", mybir.AluOpType.bypass,
                        ins=[a2a_in[pair].opt()], outs=[a2a_out[pair].opt()],
                        replica_groups=[list(range(NCORES))])

            # ---------------- Stage 2 persistent tiles
            with (
                tc.tile_pool(name="attn_out", bufs=1) as aout,
                tc.tile_pool(name="qkvres", bufs=1) as res,
            ):
                # kall8 [p, blk(4 nope heads + 1 pe), (lo,hi), T]
                # qall8 [p, blk(4 nope + 4 pe), (hi,lo), T]
                kall8 = res.tile([128, 5, 2, T], F8, tag="kall8")
                qall8 = res.tile([128, 8, T], F8, tag="qall8")
                attnT8 = aout.tile([128, HPC, 2, T], F8, tag="attnT8")
                v_sb = res.tile([128, QTILES, HPC * DV], F16, tag="v_sb")
                nc.vector.memset(kall8[64:128, 4, :, :], 0.0)

                # ---- Stage 2a: k/v expansion + score operand quantize
                with (
                    tc.tile_pool(name="proj", bufs=1) as proj,
                    tc.tile_pool(name="projs", bufs=4) as projs,
                    tc.tile_pool(name="kvps", bufs=4, space="PSUM") as kvps,
                ):
                    wkvb8_sb = proj.tile([128, 4, 2, HPC * 256], F8,
                                         tag="wkvb8")
                    nc.sync.dma_start(wkvb8_sb[:], wkvb8_d[:])
                    kvaT8 = proj.tile([128, 4, 2, T], F8, tag="kvaT8")
                    for j in range(4):
                        for hl in range(2):
                            srcg = bass.AP(
                                tensor=ag2_out.tensor,
                                offset=ag2_out.offset
                                + (j * 128 + hl * KL) * TPC,
                                ap=[[TPC, 128],
                                    [2 * (KL + DR) * TPC, NCORES], [1, TPC]])
                            eng = nc.sync if j % 2 == 0 else nc.scalar
                            eng.dma_start(
                                kvaT8[:, j, hl, :].rearrange(
                                    "p (r t) -> p r t", r=NCORES), srcg)
                    for hl in range(2):
                        srcg = bass.AP(
                            tensor=ag2_out.tensor,
                            offset=ag2_out.offset + (2 * KL + hl * 64) * TPC,
                            ap=[[TPC, 64],
                                [2 * (KL + DR) * TPC, NCORES], [1, TPC]])
                        nc.scalar.dma_start(
                            kall8[0:64, 4, hl, :].rearrange(
                                "p (r t) -> p r t", r=NCORES), srcg)
                    wo8_sb = res.tile([128, HPC, 2, HID], F8, tag="wo8")
                    nc.sync.dma_start(wo8_sb[:], wo8_d[:])

                    # K expansion: out [128 nope-dims, keys]
                    for h in range(HPC):
                        for n4 in range(4):
                            pk = kvps.tile([128, 512], F32, tag="kps")
                            for cp in range(2):
                                lhs = sap(wkvb8_sb,
                                          (2 * cp) * 2048 + 1024 + h * 128,
                                          [[2048, 2], [1, 128]])
                                rhs = sap(kvaT8, (2 * cp) * 2 * T + n4 * 512,
                                          [[2 * T, 2], [1, 512]])
                                nc.tensor.matmul(pk[:], lhs, rhs,
                                                 start=(cp == 0), stop=False,
                                                 perf_mode=DRM)
                            for c in range(4):
                                lhs = sap(wkvb8_sb, c * 2048 + h * 128,
                                          [[1024, 2], [1, 128]])
                                rhs = sap(kvaT8, c * 2 * T + n4 * 512,
                                          [[T, 2], [1, 512]])
                                nc.tensor.matmul(pk[:], lhs, rhs, start=False,
                                                 stop=(c == 3),
                                                 perf_mode=DRM)
                            kt16 = projs.tile([128, 512], F16, tag="kt16",
                                              name="kt16")
                            nc.vector.tensor_scalar_mul(kt16[:], pk[:],
                                                        float(ckt))
                            nc.vector.tensor_copy(
                                kall8[:, h, 1, n4 * 512:(n4 + 1) * 512],
                                kt16[:])
                            nc.vector.tensor_sub(
                                kall8[:, h, 0, n4 * 512:(n4 + 1) * 512],
                                kt16[:],
                                kall8[:, h, 1, n4 * 512:(n4 + 1) * 512])

                    # V expansion: out [128 tokens, v cols]
                    for hp in range(2):
                        for tt16 in range(QTILES):
                            pv = kvps.tile([128, 256], F32, tag="vps")
                            for cp in range(2):
                                lhs = sap(kvaT8,
                                          (2 * cp) * 2 * T + tt16 * 128,
                                          [[2 * T, 2], [1, 128]])
                                rhs = sap(wkvb8_sb,
                                          (2 * cp) * 2048 + 1024 + 512
                                          + hp * 256,
                                          [[2048, 2], [1, 256]])
                                nc.tensor.matmul(pv[:], lhs, rhs,
                                                 start=(cp == 0), stop=False,
                                                 perf_mode=DRM)
                            for c in range(4):
                                lhs = sap(kvaT8, c * 2 * T + tt16 * 128,
                                          [[T, 2], [1, 128]])
                                rhs = sap(wkvb8_sb,
                                          c * 2048 + 512 + hp * 256,
                                          [[1024, 2], [1, 256]])
                                nc.tensor.matmul(pv[:], lhs, rhs, start=False,
                                                 stop=(c == 3),
                                                 perf_mode=DRM)
                            nc.vector.tensor_scalar_mul(
                                v_sb[:, tt16, hp * 256:(hp + 1) * 256],
                                pv[:], float(cv))

                # ---- Stage 2c: causal attention, S^T formulation
                with (
                    tc.tile_pool(name="atw", bufs=2) as atw,
                    tc.tile_pool(name="atp", bufs=2) as atp,
                    tc.tile_pool(name="atn", bufs=2) as atn,
                    tc.tile_pool(name="apr", bufs=2) as apr,
                    tc.tile_pool(name="atps", bufs=4, space="PSUM") as atps,
                    tc.tile_pool(name="atpsA", bufs=2, space="PSUM") as atpsA,
                ):
                    def assemble_pair(pr):
                        for e in range(2):
                            hh = pr * 2 + e
                            srcq = bass.AP(
                                tensor=a2a_out[pr].tensor,
                                offset=a2a_out[pr].offset + e * 128 * TPC,
                                ap=[[TPC, 128],
                                    [384 * TPC, NCORES], [1, TPC]])
                            nc.sync.dma_start(
                                qall8[:, hh, :].rearrange(
                                    "p (s t) -> p s t", s=NCORES), srcq)
                            for half in range(2):
                                srcp2 = bass.AP(
                                    tensor=a2a_out[pr].tensor,
                                    offset=a2a_out[pr].offset
                                    + (256 + half * 64 + e * 32) * TPC,
                                    ap=[[TPC, 32],
                                        [384 * TPC, NCORES], [1, TPC]])
                                nc.sync.dma_start(
                                    qall8[half * 32:(half + 1) * 32,
                                          4 + hh, :].rearrange(
                                        "p (s t) -> p s t", s=NCORES),
                                    srcp2)

                    def attn_group(h, g, praw):
                        """scores + PV + rowsums for (head, query group).
                        Returns rsT4 (PSUM [128,4] denominators)."""
                        PT_g = atw.tile([128, QTILES, 512], F16, tag="PTg")
                        nsc = 4 * g + 4
                        for sc in range(nsc):
                            kk = sc - 4 * g
                            v0 = 128 * kk if kk >= 0 else 0
                            nq = 512 - v0
                            q0 = g * 512 + v0
                            pS = atps.tile([128, 512], F32, tag="Sps")
                            rhs = sap(qall8, h * T + q0,
                                      [[4 * T, 2], [1, nq]])
                            lhs = sap(kall8, h * 2 * T + T + sc * 128,
                                      [[(4 - h) * 2 * T, 2], [1, 128]])
                            nc.tensor.matmul(pS[:, v0:512], lhs, rhs,
                                             start=True, stop=False,
                                             perf_mode=DRM)
                            lhs = sap(kall8, h * 2 * T + sc * 128,
                                      [[(4 - h) * 2 * T, 2], [1, 128]])
                            nc.tensor.matmul(pS[:, v0:512], lhs, rhs,
                                             start=False, stop=True,
                                             perf_mode=DRM)
                            if kk >= 0:
                                nc.vector.tensor_add(pS[:, v0:v0 + 128],
                                                     pS[:, v0:v0 + 128],
                                                     triT_sb[:])
                            nc.scalar.activation(
                                PT_g[:, sc, v0:512], pS[:, v0:512],
                                AF.Exp, bias=ebias_sb[:],
                                scale=float(SM_SCALE / (SA * SA)))
                        pA4 = atpsA.tile([128, 512], F32, tag="pA4")
                        rsT4 = atpsA.tile([128, 4], F32, tag="rsT4")
                        for sc in range(nsc):
                            kk = sc - 4 * g
                            v0 = 128 * kk if kk >= 0 else 0
                            nc.tensor.matmul(
                                pA4[:, v0:512],
                                v_sb[:, sc, h * 128:(h + 1) * 128],
                                PT_g[:, sc, v0:512],
                                start=(sc == 0), stop=(sc == nsc - 1),
                                skip_group_check=True)
                            for qq in range(max(0, kk), 4):
                                nc.tensor.matmul(
                                    rsT4[:, qq:qq + 1],
                                    PT_g[:, sc, qq * 128:(qq + 1) * 128],
                                    ones16[:],
                                    start=(sc == 0 and qq == max(0, kk)),
                                    stop=(sc == nsc - 1 and qq == 3),
                                    skip_group_check=True)
                        nc.vector.tensor_copy(
                            praw[:, g * 512:(g + 1) * 512], pA4[:])
                        return rsT4

                    def norm_group(h, g, praw, rsT4):
                        rec32 = atn.tile([128, 4], F32, tag="rec32",
                                         name="rec32")
                        nc.vector.reciprocal(rec32[:], rsT4[:])
                        rec16 = atn.tile([128, 4], F16, tag="rec16",
                                         name="rec16")
                        nc.scalar.activation(rec16[:], rec32[:], AF.Copy,
                                             scale=float(SAT))
                        rscrg = dram2.tile([1, 512], F16, tag="rscrg")
                        dstg = bass.AP(tensor=rscrg.tensor,
                                       offset=rscrg.offset,
                                       ap=[[1, 128], [128, 4]])
                        nc.gpsimd.dma_start(dstg, rec16[:])
                        rrecg = atn.tile([128, 512], F16, tag="rrecg",
                                         name="rrecg")
                        bcg = bass.AP(tensor=rscrg.tensor,
                                      offset=rscrg.offset,
                                      ap=[[0, 128], [1, 512]])
                        nc.gpsimd.dma_start(rrecg[:], bcg)
                        sl_g = praw[:, g * 512:(g + 1) * 512]
                        nc.vector.tensor_mul(sl_g, sl_g, rrecg[:])
                        nc.scalar.copy(
                            attnT8[:, h, 0, g * 512:(g + 1) * 512], sl_g)
                        nc.vector.tensor_sub(
                            attnT8[:, h, 1, g * 512:(g + 1) * 512], sl_g,
                            attnT8[:, h, 0, g * 512:(g + 1) * 512])

                    def oproj_m(m):
                        orow = atw.tile([128, HID], F16, tag="orow",
                                        name="orow")
                        for n10 in range(10):
                            po = atps.tile([128, 512], F32, tag="Sps",
                                           name="Sps")
                            for jp in range(2):
                                lhs = sap(attnT8, (2 * jp) * 2 * T + m * 128,
                                          [[2 * T, 2], [1, 128]])
                                rhs = sap(wo8_sb,
                                          (2 * jp) * 2 * HID + HID
                                          + n10 * 512,
                                          [[2 * HID, 2], [1, 512]])
                                nc.tensor.matmul(po[:], lhs, rhs,
                                                 start=(jp == 0), stop=False,
                                                 perf_mode=DRM)
                            for j in range(HPC):
                                lhs = sap(attnT8, j * 2 * T + m * 128,
                                          [[T, 2], [1, 128]])
                                rhs = sap(wo8_sb, j * 2 * HID + n10 * 512,
                                          [[HID, 2], [1, 512]])
                                nc.tensor.matmul(po[:], lhs, rhs, start=False,
                                                 stop=(j == HPC - 1),
                                                 perf_mode=DRM)
                            if n10 % 2 == 0:
                                nc.scalar.activation(orow[:, n10 * 512:
                                                          (n10 + 1) * 512],
                                                     po[:], AF.Copy,
                                                     scale=float(co))
                            else:
                                nc.vector.tensor_scalar_mul(
                                    orow[:, n10 * 512:(n10 + 1) * 512],
                                    po[:], float(co))
                        nsplit = 4 if m == QTILES - 1 else 2
                        step = HID // nsplit
                        for sp_i in range(nsplit):
                            nc.sync.dma_start(
                                out_d[m * 128:(m + 1) * 128,
                                      sp_i * step:(sp_i + 1) * step],
                                orow[:, sp_i * step:(sp_i + 1) * step])

                    # heads 0,1: head-wise, one normalize per head
                    assemble_pair(0)
                    for h in range(2):
                        praw = apr.tile([128, T], F16, tag="praw",
                                        name="praw")
                        rsums = atp.tile([128, QTILES], F32, tag="rsums")
                        for g in range(4):
                            rsT4 = attn_group(h, g, praw)
                            nc.vector.tensor_copy(
                                rsums[:, g * 4:(g + 1) * 4], rsT4[:])
                        rec32f = atn.tile([128, QTILES], F32,
                                          tag="rec32f", name="rec32f")
                        nc.vector.reciprocal(rec32f[:], rsums[:])
                        rec16f = atn.tile([128, QTILES], F16,
                                          tag="rec16f", name="rec16f")
                        nc.scalar.activation(rec16f[:], rec32f[:],
                                             AF.Copy, scale=float(SAT))
                        rscr = dram2.tile([1, T], F16, tag="rscr")
                        dstr = bass.AP(tensor=rscr.tensor,
                                       offset=rscr.offset,
                                       ap=[[1, 128], [128, QTILES]])
                        nc.gpsimd.dma_start(dstr, rec16f[:])
                        rrec = atn.tile([128, T], F16, tag="rrec",
                                        name="rrec")
                        bcast = bass.AP(tensor=rscr.tensor,
                                        offset=rscr.offset,
                                        ap=[[0, 128], [1, T]])
                        nc.gpsimd.dma_start(rrec[:], bcast)
                        nc.vector.tensor_mul(praw[:], praw[:], rrec[:])
                        nc.scalar.copy(attnT8[:, h, 0, :], praw[:])
                        nc.vector.tensor_sub(attnT8[:, h, 1, :], praw[:],
                                             attnT8[:, h, 0, :])

                    # heads 2,3: group-wise, o_proj interleaved so the
                    # output writes overlap the attention tail
                    assemble_pair(1)
                    praw2 = apr.tile([128, T], F16, tag="praw", name="praw")
                    praw3 = apr.tile([128, T], F16, tag="praw", name="praw")
                    for g in range(4):
                        rs2 = attn_group(2, g, praw2)
                        norm_group(2, g, praw2, rs2)
                        if g > 0:
                            for m in range(4 * (g - 1), 4 * (g - 1) + 2):
                                oproj_m(m)
                        rs3 = attn_group(3, g, praw3)
                        norm_group(3, g, praw3, rs3)
                        if g > 0:
                            for m in range(4 * (g - 1) + 2, 4 * g):
                                oproj_m(m)
                    for m in range(12, 16):
                        oproj_m(m)

    nc.compile()
    _PROGRAM_CACHE[key] = nc
    return nc


def _host_prep(inputs):
    pos = np.asarray(inputs["positions"]).astype(np.float32)
    inv_freq = 1.0 / (THETA ** (np.arange(0, DR, 2, dtype=np.float32) / DR))
    freqs = pos[:, None] * inv_freq[None, :]
    cos, sin = np.cos(freqs), np.sin(freqs)

    eo = np.concatenate([np.arange(0, DR, 2), np.arange(1, DR, 2)])
    w_qkv_a = np.asarray(inputs["w_qkv_a"], np.float32)
    wa_cols = np.concatenate([
        w_qkv_a[:, QL:QL + KL],
        w_qkv_a[:, :QL],
    ], axis=1)
    wape_cols = w_qkv_a[:, QL + KL:][:, eo]
    w_q_b = np.asarray(inputs["w_q_b"], np.float32) * np.asarray(
        inputs["q_a_ln_w"], np.float32)[:, None]
    w_kv_b = np.asarray(inputs["w_kv_b"], np.float32) * np.asarray(
        inputs["kv_a_ln_w"], np.float32)[:, None]
    w_o = np.asarray(inputs["w_o"], np.float32)
    hid = np.asarray(inputs["hidden_states"], np.float32)
    hidT = np.ascontiguousarray(hid.T)

    scales = {
        "sh": _pow2(96.0, np.abs(hid).max()),
        "swa": _pow2(96.0, max(np.abs(wa_cols).max(),
                               np.abs(wape_cols).max())),
        "swqn": 1.0, "swqp": 1.0,
        "swkvb": _pow2(96.0, np.abs(w_kv_b).max()),
        "swo": _pow2(96.0, np.abs(w_o).max()),
    }

    # q_b column permutations (as v2), then scale + hi/lo split
    nope = w_q_b.reshape(QL, H, 192)[:, :, :DN]
    peh = w_q_b.reshape(QL, H, 192)[:, :, DN:]
    wqbn_cols = np.zeros((QL, H * DN), np.float32)
    wqbp_cols = np.zeros((QL, H * DR), np.float32)
    for d in range(NCORES):
        for hl in range(HPC):
            head = d * HPC + hl
            pair, e = hl // 2, hl % 2
            c0 = pair * 2048 + d * 256 + e * 128
            wqbn_cols[:, c0:c0 + 128] = nope[:, head, :]
            pE = peh[:, head, eo[:32]]
            pO = peh[:, head, eo[32:]]
            e0 = pair * 1024 + d * 64 + e * 32
            wqbp_cols[:, e0:e0 + 32] = pE
            wqbp_cols[:, 512 + e0:512 + e0 + 32] = pO
    scales["swqn"] = _pow2(96.0, np.abs(wqbn_cols).max())
    scales["swqp"] = _pow2(96.0, np.abs(wqbp_cols).max())

    def wsplit(wcols, s, kt):
        # [K, N] -> [128, kt, 2(lo,hi), N]
        hi, lo = _hilo(wcols * s)
        out = np.zeros((128, kt, 2, wcols.shape[1]), E4)
        for j in range(kt):
            out[:, j, 0, :] = lo[j * 128:(j + 1) * 128, :]
            out[:, j, 1, :] = hi[j * 128:(j + 1) * 128, :]
        return out

    wa8 = wsplit(wa_cols, scales["swa"], 40)
    wape8 = wsplit(wape_cols, scales["swa"], 40)
    wqbn8 = wsplit(wqbn_cols, scales["swqn"], 12)
    # wqbp8: chunk-contiguous [128, 12, 8(chunk), 2(lo,hi), 256]
    bp = wsplit(wqbp_cols, scales["swqp"], 12)      # [128, 12, 2, 2048]
    wqbp8 = np.ascontiguousarray(
        bp.reshape(128, 12, 2, 8, 256).transpose(0, 1, 3, 2, 4))

    c1 = scales["sh"] * scales["swa"]
    cosr = np.tile(cos, (1, 16)) * (SA / (SQA * scales["swqp"]))
    sinr = np.tile(sin, (1, 16)) * (SA / (SQA * scales["swqp"]))
    ctok = cos * (SA / c1)
    stok = sin * (SA / c1)
    triT = np.tril(np.full((128, 128), NEG, np.float32), -1)

    in_maps = []
    for c in range(NCORES):
        hs = [HPC * c + i for i in range(HPC)]
        kcols = np.concatenate(
            [w_kv_b[:, h * 256:h * 256 + DN] for h in hs], axis=1)
        vcols = np.concatenate(
            [w_kv_b[:, h * 256 + DN:(h + 1) * 256] for h in hs], axis=1)
        wkvb8 = wsplit(np.concatenate([kcols, vcols], axis=1),
                       scales["swkvb"], 4)
        # wo: [p(dv dim), head, 2(lo,hi), HID]
        wo_hi, wo_lo = _hilo(np.stack(
            [w_o[h * DV:(h + 1) * DV, :] for h in hs], axis=1)
            * scales["swo"])
        wo8 = np.stack([wo_lo, wo_hi], axis=2)

        sl = slice(c * TPC, (c + 1) * TPC)
        h_hi, h_lo = _hilo(np.ascontiguousarray(hidT[:, sl]) * scales["sh"])
        hT8 = np.zeros((128, 40, 2, TPC), E4)
        for j in range(40):
            hT8[:, j, 0, :] = h_hi[j * 128:(j + 1) * 128, :]
            hT8[:, j, 1, :] = h_lo[j * 128:(j + 1) * 128, :]

        in_maps.append({
            "hT8": hT8,
            "wa8": wa8,
            "wape8": wape8,
            "wqbn8": wqbn8,
            "wqbp8": wqbp8,
            "wkvb8": wkvb8,
            "wo8": np.ascontiguousarray(wo8),
            "ctok": np.ascontiguousarray(ctok[sl]).astype(np.float16),
            "stok": np.ascontiguousarray(stok[sl]).astype(np.float16),
            "cosr": np.ascontiguousarray(cosr[sl]).astype(np.float16),
            "sinr": np.ascontiguousarray(sinr[sl]).astype(np.float16),
            "triT": triT,
        })
    return in_maps, scales


def kernel(**inputs) -> np.ndarray:
    in_maps, scales = _host_prep(inputs)
    nc = build_program(scales)
    res = run_bass_kernel_spmd(nc, in_maps, core_ids=list(range(NCORES)))
    out = np.zeros((T, HID), np.float32)
    for r in res.results:
        out += r["out"].astype(np.float32)
    return out


if __name__ == "__main__":
    ins, scales = _host_prep({
        "positions": np.arange(T),
        "hidden_states": np.random.randn(T, HID).astype(np.float32),
        "w_qkv_a": np.random.randn(HID, CW).astype(np.float32) * HID ** -0.5,
        "q_a_ln_w": np.ones(QL, np.float32),
        "w_q_b": np.random.randn(QL, H * 192).astype(np.float32) * QL ** -0.5,
        "kv_a_ln_w": np.ones(KL, np.float32),
        "w_kv_b": np.random.randn(KL, H * 256).astype(np.float32) * KL ** -0.5,
        "w_o": np.random.randn(H * DV, HID).astype(np.float32) * (H * DV) ** -0.5,
    })
    build_program(scales)
    print("program built ok")


# revision 38
# speedup vs baseline: 1.0110x; 1.0110x over previous
"""DeepSeek-V2 MLA decoder layer (prefill, T=2048) on 8 Trainium2 NeuronCores.

v3: all big GEMMs (qkv_a, q_b, kv_b, scores, o_proj) run as 3-term
compensated e4m3 DoubleRow matmuls: X ~= Xhi+Xlo, W ~= Whi+Wlo (host- or
device-split, power-of-2 scaled), out = M1 + M2 where
  M1 = sum_k Xhi[k] Whi[k]      (DR over k-tile pairs, 0.25x f16 cost)
  M2 = sum_k (Xhi[k] Wlo[k] + Xlo[k] Whi[k])   (one DR per k-tile, 0.5x)
PV stays f16.  Collectives carry hi+lo fp8 pairs (same bytes as f16).
Overall structure as v2: token-parallel stage 1 with early kv-latent
AllGather and per-head-pair q AllToAll; head-parallel causal attention
in the S^T formulation; partial o_proj per core, host sums.
"""
import numpy as np
import ml_dtypes

import concourse.bass as bass
import concourse.mybir as mybir
import concourse.tile as tile
from concourse import bacc
from concourse.bass_utils import run_bass_kernel_spmd
from concourse.masks import make_identity

F16 = mybir.dt.float16
F32 = mybir.dt.float32
F8 = mybir.dt.float8e4
E4 = ml_dtypes.float8_e4m3
AX = mybir.AxisListType
AF = mybir.ActivationFunctionType
DRM = mybir.MatmulPerfMode.DoubleRow

NCORES = 8
T, HID, H = 2048, 5120, 32
DN, DR, DV, QL, KL = 128, 64, 128, 1536, 512
EPS = 1e-6
THETA = 10000.0
HPC = H // NCORES            # 4 heads per core
TPC = T // NCORES            # 256 tokens per core
CW = QL + KL + DR            # 2112
SM_SCALE = float((DN + DR) ** -0.5)
EXP_BIAS = float(-7.0 * np.log(2.0))
NEG = -1e9
QTILES = T // 128            # 16

# fixed power-of-2 scales for device-quantized activations
SQA = 16.0    # q_a latent
SKV = 16.0    # kv latent
SA = 16.0     # q/k score operands
SAT = 16.0    # attn output

_PROGRAM_CACHE = {}


def _pow2(target, amax):
    return float(2.0 ** np.floor(np.log2(target / max(amax, 1e-30))))


def _hilo(x):
    hi = x.astype(E4)
    lo = (x - hi.astype(np.float32)).astype(E4)
    return hi, lo


def sap(t, off, dims, p=128):
    fa = t[:]
    return bass.AP(tensor=fa.tensor, offset=fa.offset + off,
                   ap=[[fa.ap[0][0], p]] + [list(d) for d in dims])


def build_program(scales):
    key = tuple(sorted(scales.items()))
    if key in _PROGRAM_CACHE:
        return _PROGRAM_CACHE[key]
    c1 = scales["sh"] * scales["swa"]          # qkv_a psum scale
    cqn = SA / (SQA * scales["swqn"])          # q nope psum -> wire scale
    ckt = SA / (SKV * scales["swkvb"])         # k psum -> score scale
    cv = 1.0 / (SKV * scales["swkvb"])         # v psum -> true scale
    co = 1.0 / (SAT * scales["swo"])           # o_proj psum -> true scale
    inv_c1 = 1.0 / c1

    nc = bacc.Bacc("TRN2", target_bir_lowering=False, debug=False,
                   num_devices=NCORES)

    hT8_d = nc.dram_tensor("hT8", [128, 40, 2, TPC], F8,
                           kind="ExternalInput").ap()
    wa8_d = nc.dram_tensor("wa8", [128, 40, 2, CW - DR], F8,
                           kind="ExternalInput").ap()
    wape8_d = nc.dram_tensor("wape8", [128, 40, 2, DR], F8,
                             kind="ExternalInput").ap()
    wqbn8_d = nc.dram_tensor("wqbn8", [128, 12, 2, H * DN], F8,
                             kind="ExternalInput").ap()
    wqbp8_d = nc.dram_tensor("wqbp8", [128, 12, 8, 2, 256], F8,
                             kind="ExternalInput").ap()
    wkvb8_d = nc.dram_tensor("wkvb8", [128, 4, 2, HPC * 256], F8,
                             kind="ExternalInput").ap()
    wo8_d = nc.dram_tensor("wo8", [128, HPC, 2, HID], F8,
                           kind="ExternalInput").ap()
    ctok_d = nc.dram_tensor("ctok", [TPC, 32], F16, kind="ExternalInput").ap()
    stok_d = nc.dram_tensor("stok", [TPC, 32], F16, kind="ExternalInput").ap()
    cosr_d = nc.dram_tensor("cosr", [TPC, 512], F16, kind="ExternalInput").ap()
    sinr_d = nc.dram_tensor("sinr", [TPC, 512], F16, kind="ExternalInput").ap()
    triT_d = nc.dram_tensor("triT", [128, 128], F32, kind="ExternalInput").ap()
    out_d = nc.dram_tensor("out", [T, HID], F16, kind="ExternalOutput").ap()

    with tile.TileContext(nc) as tc:
        with (
            tc.tile_pool(name="const", bufs=1) as cst,
            tc.tile_pool(name="dram", bufs=1, space="DRAM") as dram,
            tc.tile_pool(name="dram2", bufs=4, space="DRAM") as dram2,
            tc.tile_pool(name="proje", bufs=1) as proje,
        ):
            ident16 = cst.tile([128, 128], F16, tag="id16")
            make_identity(nc, ident16[:])
            ones16 = cst.tile([128, 1], F16, tag="ones16")
            nc.vector.memset(ones16[:], 1.0)
            triT_sb = cst.tile([128, 128], F32, tag="triT")
            nc.gpsimd.dma_start(triT_sb[:], triT_d[:])
            ctok_sb = cst.tile([128, 2, 32], F16, tag="ctok")
            nc.gpsimd.dma_start(ctok_sb[:], ctok_d.rearrange("(a p) f -> p a f", p=128))
            stok_sb = cst.tile([128, 2, 32], F16, tag="stok")
            nc.gpsimd.dma_start(stok_sb[:], stok_d.rearrange("(a p) f -> p a f", p=128))
            cosr_sb = cst.tile([128, 2, 512], F16, tag="cosr")
            nc.gpsimd.dma_start(cosr_sb[:], cosr_d.rearrange("(a p) f -> p a f", p=128))
            sinr_sb = cst.tile([128, 2, 512], F16, tag="sinr")
            nc.gpsimd.dma_start(sinr_sb[:], sinr_d.rearrange("(a p) f -> p a f", p=128))
            epss_sb = cst.tile([128, 1], F32, tag="epss")
            nc.vector.memset(epss_sb[:], float(EPS / (SQA * SQA)))
            ebias_sb = cst.tile([128, 1], F32, tag="ebias")
            nc.vector.memset(ebias_sb[:], EXP_BIAS)
            warm = cst.tile([128, 1], F32, tag="warm")
            nc.vector.memset(warm[:], 1.0)
            wsink = cst.tile([128, 4], F32, tag="wsink")
            nc.scalar.activation(wsink[:, 0:1], warm[:], AF.Square)
            nc.scalar.activation(wsink[:, 1:2], warm[:], AF.Sqrt)
            nc.scalar.activation(wsink[:, 2:3], warm[:], AF.Exp)
            nc.scalar.activation(wsink[:, 3:4], warm[:], AF.Copy)

            ag2_in = dram.tile([2 * (KL + DR), TPC], F8, tag="ag2in")
            ag2_out = dram.tile([NCORES * 2 * (KL + DR), TPC], F8,
                                addr_space="Shared", tag="ag2out")
            a2a_in = [dram.tile([NCORES * 384, TPC], F8, tag=f"a2ain{p}",
                                name=f"a2ain{p}") for p in range(2)]
            a2a_out = [dram.tile([NCORES * 384, TPC], F8, tag=f"a2aout{p}",
                                 name=f"a2aout{p}") for p in range(2)]

            # ---------------- Stage 1
            with (
                tc.tile_pool(name="ph1", bufs=1) as ph1,
                tc.tile_pool(name="ph1w", bufs=4) as ph1w,
                tc.tile_pool(name="ph1pe", bufs=1) as ph1pe,
                tc.tile_pool(name="ph1qw", bufs=3) as ph1qw,
                tc.tile_pool(name="ph1s", bufs=4) as ph1s,
                tc.tile_pool(name="ph1r", bufs=1) as ph1r,
                tc.tile_pool(name="ph1n", bufs=3) as ph1n,
                tc.tile_pool(name="ph1ps", bufs=2, space="PSUM") as ph1ps,
                tc.tile_pool(name="ph1tp", bufs=2, space="PSUM") as ph1tp,
                tc.tile_pool(name="ph1qps", bufs=2, space="PSUM") as ph1qps,
            ):
                stage = [ph1.tile([128, CW], F16, tag=f"stage{tt}",
                                  name=f"stage{tt}") for tt in range(2)]
                hT8_sb = ph1.tile([128, 40, 2, TPC], F8, tag="hT8")
                for kg in range(4):
                    nc.scalar.dma_start(
                        hT8_sb[:, kg * 10:(kg + 1) * 10, :, :],
                        hT8_d[:, kg * 10:(kg + 1) * 10, :, :])

                # x3 DR matmul emission for qkv_a.  hT8_sb layout
                # [p, j(40), t(256), hilo(2)]; wa_t [p, jl(8), (lo,hi), w].
                def qkv_x3(n0, w, kv=False):
                    ps = [ph1ps.tile([128, w], F32, tag=f"s1ps{tt}",
                                     name=f"s1ps{tt}") for tt in range(2)]
                    for kg in range(5):
                        wa_t = ph1w.tile([128, 8, 2, w], F8, tag="wa_t",
                                         name="wa_t")
                        nc.sync.dma_start(
                            wa_t[:], wa8_d[:, kg * 8:(kg + 1) * 8, :,
                                           n0:n0 + w])
                        for tt in range(2):
                            for jp in range(4):
                                j = kg * 8 + 2 * jp
                                lhs = sap(hT8_sb, j * 512 + tt * 128,
                                          [[512, 2], [1, 128]])
                                rhs = sap(wa_t, (2 * jp) * 2 * w + w,
                                          [[2 * w, 2], [1, w]])
                                nc.tensor.matmul(
                                    ps[tt][:], lhs, rhs,
                                    start=(kg == 0 and jp == 0), stop=False,
                                    perf_mode=DRM)
                            for jl in range(8):
                                j = kg * 8 + jl
                                lhs = sap(hT8_sb, j * 512 + tt * 128,
                                          [[256, 2], [1, 128]])
                                rhs = sap(wa_t, jl * 2 * w,
                                          [[w, 2], [1, w]])
                                nc.tensor.matmul(
                                    ps[tt][:], lhs, rhs, start=False,
                                    stop=(kg == 4 and jl == 7),
                                    perf_mode=DRM)
                    return ps

                # pe slice: all 40 k-tiles in one weight load
                def qkv_x3_pe():
                    w = DR
                    ps = [ph1ps.tile([128, w], F32, tag=f"s1ps{tt}",
                                     name=f"s1ps{tt}") for tt in range(2)]
                    wa_t = ph1pe.tile([128, 40, 2, w], F8, tag="wa_pe")
                    nc.sync.dma_start(wa_t[:], wape8_d[:])
                    for tt in range(2):
                        for jp in range(20):
                            lhs = sap(hT8_sb, (2 * jp) * 512 + tt * 128,
                                      [[512, 2], [1, 128]])
                            rhs = sap(wa_t, (2 * jp) * 2 * w + w,
                                      [[2 * w, 2], [1, w]])
                            nc.tensor.matmul(ps[tt][:], lhs, rhs,
                                             start=(jp == 0), stop=False,
                                             perf_mode=DRM)
                        for j in range(40):
                            lhs = sap(hT8_sb, j * 512 + tt * 128,
                                      [[256, 2], [1, 128]])
                            rhs = sap(wa_t, j * 2 * w, [[w, 2], [1, w]])
                            nc.tensor.matmul(ps[tt][:], lhs, rhs, start=False,
                                             stop=(j == 39), perf_mode=DRM)
                    return ps

                # wa col layout: [kv 512 | pe 64 | q 1536]
                kvps = qkv_x3(0, KL, kv=True)
                peps = qkv_x3_pe()

                for tt in range(2):
                    sums = ph1s.tile([128, 4], F32, tag="s1sums")
                    dump = ph1s.tile([128, 512], F16, tag="s1dump")
                    nc.scalar.activation(dump[:], kvps[tt][:], AF.Square,
                                         scale=inv_c1,
                                         accum_out=sums[:, 3:4])
                    rkv = ph1s.tile([128, 1], F32, tag="rkv")
                    nc.scalar.activation(rkv[:], sums[:, 3:4], AF.Sqrt,
                                         bias=epss_sb[:],
                                         scale=float(1.0 / (KL * SKV * SKV)))
                    nc.vector.reciprocal(rkv[:], rkv[:])
                    # rkv = SKV / rms(x); psum = c1*x -> scale by rkv*inv_c1
                    rkv2 = ph1s.tile([128, 1], F32, tag="rkv2")
                    nc.vector.tensor_scalar_mul(rkv2[:], rkv[:],
                                                float(inv_c1))
                    kva16 = ph1.tile([128, KL], F16, tag=f"kva16_{tt}",
                                     name=f"kva16_{tt}")
                    nc.scalar.activation(kva16[:], kvps[tt][:],
                                         AF.Copy, scale=rkv2[:])
                    kpe16 = ph1.tile([128, 64], F16, tag=f"kpe16_{tt}",
                                     name=f"kpe16_{tt}")
                    pe = peps[tt][:]
                    ct, st = ctok_sb[:, tt, :], stok_sb[:, tt, :]
                    t1 = ph1s.tile([128, 32], F32, tag="rt1")
                    t2 = ph1s.tile([128, 32], F32, tag="rt2")
                    nc.vector.tensor_mul(t1[:], pe[:, 0:32], ct)
                    nc.vector.tensor_mul(t2[:], pe[:, 32:64], st)
                    nc.vector.tensor_sub(kpe16[:, 0:32], t1[:], t2[:])
                    t3 = ph1s.tile([128, 32], F32, tag="rt3")
                    t4 = ph1s.tile([128, 32], F32, tag="rt4")
                    nc.vector.tensor_mul(t3[:], pe[:, 32:64], ct)
                    nc.vector.tensor_mul(t4[:], pe[:, 0:32], st)
                    nc.vector.tensor_add(kpe16[:, 32:64], t3[:], t4[:])

                    # transpose + hi/lo quantize -> ag2_in rows
                    # [kva_hi 0-511 | kva_lo 512-1023 | kpe_lo 1024-1087 |
                    #  kpe_hi 1088-1151]
                    for b in range(4):
                        tp = ph1tp.tile([128, 128], F16, tag="s1tp",
                                        name="s1tp")
                        nc.tensor.transpose(tp[:],
                                            kva16[:, b * 128:(b + 1) * 128],
                                            ident16[:])
                        hl8 = ph1s.tile([128, 2, 128], F8, tag="kvhl",
                                        name="kvhl")
                        nc.vector.tensor_copy(hl8[:, 0, :], tp[:])
                        nc.vector.tensor_sub(hl8[:, 1, :], tp[:],
                                             hl8[:, 0, :])
                        dst = bass.AP(
                            tensor=ag2_in.tensor,
                            offset=ag2_in.offset + b * 128 * TPC + tt * 128,
                            ap=[[TPC, 128], [KL * TPC, 2], [1, 128]])
                        nc.scalar.dma_start(dst, hl8[:])
                    tp2f = ph1tp.tile([128, 128], F16, tag="s1tp", name="s1tp")
                    nc.tensor.transpose(tp2f[0:64, :], kpe16[:], ident16[:])
                    phl = ph1s.tile([64, 2, 128], F8, tag="kphl")
                    nc.vector.tensor_copy(phl[:, 1, :], tp2f[0:64, :])
                    nc.vector.tensor_sub(phl[:, 0, :], tp2f[0:64, :],
                                         phl[:, 1, :])
                    dst = bass.AP(
                        tensor=ag2_in.tensor,
                        offset=ag2_in.offset + 2 * KL * TPC + tt * 128,
                        ap=[[TPC, 64], [64 * TPC, 2], [1, 128]])
                    nc.scalar.dma_start(dst, phl[:])

                nc.gpsimd.collective_compute(
                    "AllGather", mybir.AluOpType.bypass,
                    ins=[ag2_in.opt()], outs=[ag2_out.opt()],
                    replica_groups=[list(range(NCORES))])

                # copy q slices into stage (descaled to true values)
                def stage_copy(ps, n0, w):
                    for tt in range(2):
                        if tt == 0:
                            nc.scalar.activation(stage[tt][:, n0:n0 + w],
                                                 ps[tt][:], AF.Copy,
                                                 scale=float(inv_c1))
                        else:
                            nc.vector.tensor_scalar_mul(
                                stage[tt][:, n0:n0 + w], ps[tt][:],
                                float(inv_c1))

                qps0 = qkv_x3(KL, 512)
                stage_copy(qps0, KL + DR, 512)
                qps1 = qkv_x3(KL + 512, 512)
                stage_copy(qps1, KL + DR + 512, 512)
                qps2 = qkv_x3(KL + 1024, 512)
                stage_copy(qps2, KL + DR + 1024, 512)
                KVW = KL + DR
                qa16 = [None, None]
                for tt in range(2):
                    sums = ph1s.tile([128, 4], F32, tag="s1sums")
                    dump = ph1s.tile([128, 512], F16, tag="s1dump")
                    for i in range(3):
                        nc.scalar.activation(
                            dump[:], stage[tt][:, KVW + i * 512:KVW + (i + 1) * 512],
                            AF.Square, accum_out=sums[:, i:i + 1])
                    qs = ph1s.tile([128, 1], F32, tag="qs")
                    nc.vector.reduce_sum(qs[:], sums[:, 0:3], axis=AX.X)
                    rq = ph1s.tile([128, 1], F32, tag="rq")
                    nc.scalar.activation(rq[:], qs[:], AF.Sqrt,
                                         bias=epss_sb[:],
                                         scale=float(1.0 / (QL * SQA * SQA)))
                    nc.vector.reciprocal(rq[:], rq[:])   # = SQA / rms
                    qa16[tt] = ph1.tile([128, QL], F16, tag=f"qa16_{tt}",
                                        name=f"qa16_{tt}")
                    for i in range(3):
                        nc.scalar.activation(
                            qa16[tt][:, i * 512:(i + 1) * 512],
                            stage[tt][:, KVW + i * 512:KVW + (i + 1) * 512],
                            AF.Copy, scale=rq[:])

                # q_aT hi/lo: [128, c(12), (hi,lo), 256] via PE transposes
                qaT8 = ph1.tile([128, 12, 2, TPC], F8, tag="qaT8")
                for tt in range(2):
                    for c in range(12):
                        tp = ph1tp.tile([128, 128], F16, tag="s1tp",
                                        name="s1tp")
                        nc.tensor.transpose(tp[:],
                                            qa16[tt][:, c * 128:(c + 1) * 128],
                                            ident16[:])
                        nc.vector.tensor_copy(
                            qaT8[:, c, 0, tt * 128:(tt + 1) * 128], tp[:])
                        nc.vector.tensor_sub(
                            qaT8[:, c, 1, tt * 128:(tt + 1) * 128], tp[:],
                            qaT8[:, c, 0, tt * 128:(tt + 1) * 128])

                # q_b x3 helpers: stationary = weights [p, c, (lo,hi), cols]
                def qb_mm(pq, wq8, wcols, col0, ncol):
                    for cp in range(6):
                        lhs = sap(wq8, (2 * cp) * 2 * wcols + wcols + col0,
                                  [[2 * wcols, 2], [1, ncol]])
                        rhs = sap(qaT8, (2 * cp) * 2 * TPC,
                                  [[2 * TPC, 2], [1, TPC]])
                        nc.tensor.matmul(pq[:], lhs, rhs, start=(cp == 0),
                                         stop=False, perf_mode=DRM)
                    for c in range(12):
                        lhs = sap(wq8, c * 2 * wcols + col0,
                                  [[wcols, 2], [1, ncol]])
                        rhs = sap(qaT8, c * 2 * TPC, [[TPC, 2], [1, TPC]])
                        nc.tensor.matmul(pq[:], lhs, rhs, start=False,
                                         stop=(c == 11), perf_mode=DRM)

                # wqbn col = pair*2048 + d*256 + (h%2)*128 + dn
                # wqbp col = pair*1024 + half*512 + d*64 + (h%2)*32 + f
                # a2a rows per dest: [hE_hi 128 | hO_hi 128 | peE_hi 64 |
                #   peO_hi 64 | hE_lo 128 | hO_lo 128 | peE_lo 64 | peO_lo 64]
                for pair in range(2):
                    qpe = ph1r.tile([128, 2, 1024], F32, tag=f"qpe{pair}",
                                    name=f"qpe{pair}")
                    # pe: token-stationary, out [128 tok, 256 pe cols]
                    for sg8 in range(4):
                        wp8 = ph1qw.tile([128, 12, 2, 256], F8, tag="wqp",
                                         name="wqp")
                        nc.sync.dma_start(
                            wp8[:], wqbp8_d[:, :, pair * 4 + sg8, :, :])
                        for tt in range(2):
                            pp = ph1qps.tile([128, 256], F32, tag="pq",
                                             name="pq")
                            for cp in range(6):
                                lhs = sap(qaT8, (2 * cp) * 512 + tt * 128,
                                          [[512, 2], [1, 128]])
                                rhs = sap(wp8, (2 * cp) * 512 + 256,
                                          [[512, 2], [1, 256]])
                                nc.tensor.matmul(pp[:], lhs, rhs,
                                                 start=(cp == 0), stop=False,
                                                 perf_mode=DRM)
                            for c in range(12):
                                lhs = sap(qaT8, c * 512 + tt * 128,
                                          [[256, 2], [1, 128]])
                                rhs = sap(wp8, c * 512,
                                          [[256, 2], [1, 256]])
                                nc.tensor.matmul(pp[:], lhs, rhs, start=False,
                                                 stop=(c == 11),
                                                 perf_mode=DRM)
                            nc.scalar.copy(
                                qpe[:, tt, sg8 * 256:(sg8 + 1) * 256], pp[:])

                    def emit_nope(mg):
                        nsb8 = ph1n.tile([128, 4, TPC], F8, tag="nsb",
                                         name="nsb")
                        wq8 = ph1qw.tile([128, 12, 2, 512], F8, tag="wqn",
                                         name="wqn")
                        nc.sync.dma_start(
                            wq8[:], wqbn8_d[:, :, :, (pair * 4 + mg) * 512:
                                            (pair * 4 + mg + 1) * 512])
                        for ml in range(4):
                            pq = ph1qps.tile([128, TPC], F32, tag="pq",
                                             name="pq")
                            qb_mm(pq, wq8, 512, ml * 128, 128)
                            nsb16 = ph1s.tile([128, TPC], F16, tag="nsb16",
                                              name="nsb16")
                            nc.scalar.activation(nsb16[:], pq[:], AF.Copy,
                                                 scale=float(cqn))
                            nc.vector.tensor_copy(nsb8[:, ml, :], nsb16[:])
                        for dl in range(2):
                            d = mg * 2 + dl
                            dst = bass.AP(
                                tensor=a2a_in[pair].tensor,
                                offset=a2a_in[pair].offset + d * 384 * TPC,
                                ap=[[TPC, 128], [128 * TPC, 2], [1, TPC]])
                            nc.scalar.dma_start(
                                dst, nsb8[:, 2 * dl:2 * dl + 2, :])

                    emit_nope(0)
                    for tt in range(2):
                        cr, sr = cosr_sb[:, tt, :], sinr_sb[:, tt, :]
                        qpe16 = ph1r.tile([128, 1024], F16, tag="qpe16",
                                          name="qpe16")
                        eE = ph1r.tile([128, 512], F32, tag="ropeE",
                                       name="ropeE")
                        eO = ph1r.tile([128, 512], F32, tag="ropeO",
                                       name="ropeO")
                        t2 = ph1r.tile([128, 512], F32, tag="ropet2",
                                       name="ropet2")
                        qq = qpe[:, tt, :]
                        nc.vector.tensor_mul(eE[:], qq[:, 0:512], cr)
                        nc.vector.tensor_mul(t2[:], qq[:, 512:1024], sr)
                        nc.vector.tensor_sub(qpe16[:, 0:512], eE[:], t2[:])
                        nc.vector.tensor_mul(eO[:], qq[:, 512:1024], cr)
                        nc.vector.tensor_mul(t2[:], qq[:, 0:512], sr)
                        nc.vector.tensor_add(qpe16[:, 512:1024], eO[:], t2[:])
                        # transpose per (half, d-pair); hi/lo -> pestg8
                        pestg8 = ph1n.tile([64, 2, 8, 128], F8,
                                           tag="pestg", name="pestg")
                        for half in range(2):
                            for d in range(0, 8, 2):
                                s0 = half * 512 + d * 64
                                tp = ph1tp.tile([128, 128], F16, tag="s1tp",
                                                name="s1tp")
                                nc.tensor.transpose(tp[:],
                                                    qpe16[:, s0:s0 + 128],
                                                    ident16[:])
                                nc.vector.tensor_copy(
                                    pestg8[:, half, d, :], tp[0:64, :])
                                nc.vector.tensor_copy(
                                    pestg8[:, half, d + 1, :],
                                    tp[64:128, :])
                        for half in range(2):
                            dst = bass.AP(
                                tensor=a2a_in[pair].tensor,
                                offset=a2a_in[pair].offset
                                + (256 + half * 64) * TPC + tt * 128,
                                ap=[[TPC, 64], [384 * TPC, 8], [1, 128]])
                            nc.scalar.dma_start(dst, pestg8[:, half, :, :])

                    for mg_i in range(1, 4):
                        emit_nope(mg_i)
                    nc.gpsimd.collective_compute(
                        "AllToAll", mybir.AluOpType.bypass,
                        ins=[a2a_in[pair].opt()], outs=[a2a_out[pair].opt()],
                        replica_groups=[list(range(NCORES))])

            # ---------------- Stage 2 persistent tiles
            with (
                tc.tile_pool(name="attn_out", bufs=1) as aout,
                tc.tile_pool(name="qkvres", bufs=1) as res,
            ):
                # kall8 [p, blk(4 nope heads + 1 pe), (lo,hi), T]
                # qall8 [p, blk(4 nope + 4 pe), (hi,lo), T]
                kall8 = res.tile([128, 5, 2, T], F8, tag="kall8")
                qall8 = res.tile([128, 8, T], F8, tag="qall8")
                attnT8 = aout.tile([128, HPC, 2, T], F8, tag="attnT8")
                v_sb = res.tile([128, QTILES, HPC * DV], F16, tag="v_sb")
                nc.vector.memset(kall8[64:128, 4, :, :], 0.0)

                # ---- Stage 2a: k/v expansion + score operand quantize
                with (
                    tc.tile_pool(name="proj", bufs=1) as proj,
                    tc.tile_pool(name="projs", bufs=4) as projs,
                    tc.tile_pool(name="kvps", bufs=4, space="PSUM") as kvps,
                ):
                    wkvb8_sb = proj.tile([128, 4, 2, HPC * 256], F8,
                                         tag="wkvb8")
                    nc.sync.dma_start(wkvb8_sb[:], wkvb8_d[:])
                    kvaT8 = proj.tile([128, 4, 2, T], F8, tag="kvaT8")
                    for j in range(4):
                        for hl in range(2):
                            srcg = bass.AP(
                                tensor=ag2_out.tensor,
                                offset=ag2_out.offset
                                + (j * 128 + hl * KL) * TPC,
                                ap=[[TPC, 128],
                                    [2 * (KL + DR) * TPC, NCORES], [1, TPC]])
                            eng = nc.sync if j % 2 == 0 else nc.scalar
                            eng.dma_start(
                                kvaT8[:, j, hl, :].rearrange(
                                    "p (r t) -> p r t", r=NCORES), srcg)
                    for hl in range(2):
                        srcg = bass.AP(
                            tensor=ag2_out.tensor,
                            offset=ag2_out.offset + (2 * KL + hl * 64) * TPC,
                            ap=[[TPC, 64],
                                [2 * (KL + DR) * TPC, NCORES], [1, TPC]])
                        nc.scalar.dma_start(
                            kall8[0:64, 4, hl, :].rearrange(
                                "p (r t) -> p r t", r=NCORES), srcg)
                    wo8_sb = res.tile([128, HPC, 2, HID], F8, tag="wo8")
                    nc.sync.dma_start(wo8_sb[:], wo8_d[:])

                    # K expansion: out [128 nope-dims, keys]
                    for h in range(HPC):
                        for n4 in range(4):
                            pk = kvps.tile([128, 512], F32, tag="kps")
                            for cp in range(2):
                                lhs = sap(wkvb8_sb,
                                          (2 * cp) * 2048 + 1024 + h * 128,
                                          [[2048, 2], [1, 128]])
                                rhs = sap(kvaT8, (2 * cp) * 2 * T + n4 * 512,
                                          [[2 * T, 2], [1, 512]])
                                nc.tensor.matmul(pk[:], lhs, rhs,
                                                 start=(cp == 0), stop=False,
                                                 perf_mode=DRM)
                            for c in range(4):
                                lhs = sap(wkvb8_sb, c * 2048 + h * 128,
                                          [[1024, 2], [1, 128]])
                                rhs = sap(kvaT8, c * 2 * T + n4 * 512,
                                          [[T, 2], [1, 512]])
                                nc.tensor.matmul(pk[:], lhs, rhs, start=False,
                                                 stop=(c == 3),
                                                 perf_mode=DRM)
                            kt16 = projs.tile([128, 512], F16, tag="kt16",
                                              name="kt16")
                            nc.vector.tensor_scalar_mul(kt16[:], pk[:],
                                                        float(ckt))
                            nc.vector.tensor_copy(
                                kall8[:, h, 1, n4 * 512:(n4 + 1) * 512],
                                kt16[:])
                            nc.vector.tensor_sub(
                                kall8[:, h, 0, n4 * 512:(n4 + 1) * 512],
                                kt16[:],
                                kall8[:, h, 1, n4 * 512:(n4 + 1) * 512])

                    # V expansion: out [128 tokens, v cols]
                    for hp in range(2):
                        for tt16 in range(QTILES):
                            pv = kvps.tile([128, 256], F32, tag="vps")
                            for cp in range(2):
                                lhs = sap(kvaT8,
                                          (2 * cp) * 2 * T + tt16 * 128,
                                          [[2 * T, 2], [1, 128]])
                                rhs = sap(wkvb8_sb,
                                          (2 * cp) * 2048 + 1024 + 512
                                          + hp * 256,
                                          [[2048, 2], [1, 256]])
                                nc.tensor.matmul(pv[:], lhs, rhs,
                                                 start=(cp == 0), stop=False,
                                                 perf_mode=DRM)
                            for c in range(4):
                                lhs = sap(kvaT8, c * 2 * T + tt16 * 128,
                                          [[T, 2], [1, 128]])
                                rhs = sap(wkvb8_sb,
                                          c * 2048 + 512 + hp * 256,
                                          [[1024, 2], [1, 256]])
                                nc.tensor.matmul(pv[:], lhs, rhs, start=False,
                                                 stop=(c == 3),
                                                 perf_mode=DRM)
                            nc.vector.tensor_scalar_mul(
                                v_sb[:, tt16, hp * 256:(hp + 1) * 256],
                                pv[:], float(cv))

                # ---- Stage 2c: causal attention, S^T formulation
                with (
                    tc.tile_pool(name="atw", bufs=2) as atw,
                    tc.tile_pool(name="atp", bufs=2) as atp,
                    tc.tile_pool(name="atn", bufs=2) as atn,
                    tc.tile_pool(name="apr", bufs=3) as apr,
                    tc.tile_pool(name="atps", bufs=4, space="PSUM") as atps,
                    tc.tile_pool(name="atpsA", bufs=2, space="PSUM") as atpsA,
                ):
                    def assemble_pair(pr):
                        for e in range(2):
                            hh = pr * 2 + e
                            srcq = bass.AP(
                                tensor=a2a_out[pr].tensor,
                                offset=a2a_out[pr].offset + e * 128 * TPC,
                                ap=[[TPC, 128],
                                    [384 * TPC, NCORES], [1, TPC]])
                            nc.sync.dma_start(
                                qall8[:, hh, :].rearrange(
                                    "p (s t) -> p s t", s=NCORES), srcq)
                            for half in range(2):
                                srcp2 = bass.AP(
                                    tensor=a2a_out[pr].tensor,
                                    offset=a2a_out[pr].offset
                                    + (256 + half * 64 + e * 32) * TPC,
                                    ap=[[TPC, 32],
                                        [384 * TPC, NCORES], [1, TPC]])
                                nc.sync.dma_start(
                                    qall8[half * 32:(half + 1) * 32,
                                          4 + hh, :].rearrange(
                                        "p (s t) -> p s t", s=NCORES),
                                    srcp2)

                    def attn_group(h, g, praw):
                        """scores + PV + rowsums for (head, query group).
                        Returns rsT4 (PSUM [128,4] denominators)."""
                        PT_g = atw.tile([128, QTILES, 512], F16, tag="PTg")
                        nsc = 4 * g + 4
                        for sc in range(nsc):
                            kk = sc - 4 * g
                            v0 = 128 * kk if kk >= 0 else 0
                            nq = 512 - v0
                            q0 = g * 512 + v0
                            pS = atps.tile([128, 512], F32, tag="Sps")
                            rhs = sap(qall8, h * T + q0,
                                      [[4 * T, 2], [1, nq]])
                            lhs = sap(kall8, h * 2 * T + T + sc * 128,
                                      [[(4 - h) * 2 * T, 2], [1, 128]])
                            nc.tensor.matmul(pS[:, v0:512], lhs, rhs,
                                             start=True, stop=False,
                                             perf_mode=DRM)
                            lhs = sap(kall8, h * 2 * T + sc * 128,
                                      [[(4 - h) * 2 * T, 2], [1, 128]])
                            nc.tensor.matmul(pS[:, v0:512], lhs, rhs,
                                             start=False, stop=True,
                                             perf_mode=DRM)
                            if kk >= 0:
                                nc.vector.tensor_add(pS[:, v0:v0 + 128],
                                                     pS[:, v0:v0 + 128],
                                                     triT_sb[:])
                            nc.scalar.activation(
                                PT_g[:, sc, v0:512], pS[:, v0:512],
                                AF.Exp, bias=ebias_sb[:],
                                scale=float(SM_SCALE / (SA * SA)))
                        pA4 = atpsA.tile([128, 512], F32, tag="pA4")
                        rsT4 = atpsA.tile([128, 4], F32, tag="rsT4")
                        for sc in range(nsc):
                            kk = sc - 4 * g
                            v0 = 128 * kk if kk >= 0 else 0
                            nc.tensor.matmul(
                                pA4[:, v0:512],
                                v_sb[:, sc, h * 128:(h + 1) * 128],
                                PT_g[:, sc, v0:512],
                                start=(sc == 0), stop=(sc == nsc - 1),
                                skip_group_check=True)
                            for qq in range(max(0, kk), 4):
                                nc.tensor.matmul(
                                    rsT4[:, qq:qq + 1],
                                    PT_g[:, sc, qq * 128:(qq + 1) * 128],
                                    ones16[:],
                                    start=(sc == 0 and qq == max(0, kk)),
                                    stop=(sc == nsc - 1 and qq == 3),
                                    skip_group_check=True)
                        nc.vector.tensor_copy(
                            praw[:, g * 512:(g + 1) * 512], pA4[:])
                        return rsT4

                    def norm_group(h, g, praw, rsT4):
                        rec32 = atn.tile([128, 4], F32, tag="rec32",
                                         name="rec32")
                        nc.vector.reciprocal(rec32[:], rsT4[:])
                        rec16 = atn.tile([128, 4], F16, tag="rec16",
                                         name="rec16")
                        nc.scalar.activation(rec16[:], rec32[:], AF.Copy,
                                             scale=float(SAT))
                        rscrg = dram2.tile([1, 512], F16, tag="rscrg")
                        dstg = bass.AP(tensor=rscrg.tensor,
                                       offset=rscrg.offset,
                                       ap=[[1, 128], [128, 4]])
                        nc.gpsimd.dma_start(dstg, rec16[:])
                        rrecg = atn.tile([128, 512], F16, tag="rrecg",
                                         name="rrecg")
                        bcg = bass.AP(tensor=rscrg.tensor,
                                      offset=rscrg.offset,
                                      ap=[[0, 128], [1, 512]])
                        nc.gpsimd.dma_start(rrecg[:], bcg)
                        sl_g = praw[:, g * 512:(g + 1) * 512]
                        nc.vector.tensor_mul(sl_g, sl_g, rrecg[:])
                        nc.scalar.copy(
                            attnT8[:, h, 0, g * 512:(g + 1) * 512], sl_g)
                        nc.vector.tensor_sub(
                            attnT8[:, h, 1, g * 512:(g + 1) * 512], sl_g,
                            attnT8[:, h, 0, g * 512:(g + 1) * 512])

                    def oproj_m(m):
                        orow = atw.tile([128, HID], F16, tag="orow",
                                        name="orow")
                        for n10 in range(10):
                            po = atps.tile([128, 512], F32, tag="Sps",
                                           name="Sps")
                            for jp in range(2):
                                lhs = sap(attnT8, (2 * jp) * 2 * T + m * 128,
                                          [[2 * T, 2], [1, 128]])
                                rhs = sap(wo8_sb,
                                          (2 * jp) * 2 * HID + HID
                                          + n10 * 512,
                                          [[2 * HID, 2], [1, 512]])
                                nc.tensor.matmul(po[:], lhs, rhs,
                                                 start=(jp == 0), stop=False,
                                                 perf_mode=DRM)
                            for j in range(HPC):
                                lhs = sap(attnT8, j * 2 * T + m * 128,
                                          [[T, 2], [1, 128]])
                                rhs = sap(wo8_sb, j * 2 * HID + n10 * 512,
                                          [[HID, 2], [1, 512]])
                                nc.tensor.matmul(po[:], lhs, rhs, start=False,
                                                 stop=(j == HPC - 1),
                                                 perf_mode=DRM)
                            if n10 % 2 == 0:
                                nc.scalar.activation(orow[:, n10 * 512:
                                                          (n10 + 1) * 512],
                                                     po[:], AF.Copy,
                                                     scale=float(co))
                            else:
                                nc.vector.tensor_scalar_mul(
                                    orow[:, n10 * 512:(n10 + 1) * 512],
                                    po[:], float(co))
                        nsplit = 4 if m == QTILES - 1 else 2
                        step = HID // nsplit
                        for sp_i in range(nsplit):
                            nc.sync.dma_start(
                                out_d[m * 128:(m + 1) * 128,
                                      sp_i * step:(sp_i + 1) * step],
                                orow[:, sp_i * step:(sp_i + 1) * step])

                    # heads 0,1: head-wise, one normalize per head
                    assemble_pair(0)
                    for h in range(2):
                        praw = apr.tile([128, T], F16, tag="praw",
                                        name="praw")
                        rsums = atp.tile([128, QTILES], F32, tag="rsums")
                        for g in range(4):
                            rsT4 = attn_group(h, g, praw)
                            nc.vector.tensor_copy(
                                rsums[:, g * 4:(g + 1) * 4], rsT4[:])
                        rec32f = atn.tile([128, QTILES], F32,
                                          tag="rec32f", name="rec32f")
                        nc.vector.reciprocal(rec32f[:], rsums[:])
                        rec16f = atn.tile([128, QTILES], F16,
                                          tag="rec16f", name="rec16f")
                        nc.scalar.activation(rec16f[:], rec32f[:],
                                             AF.Copy, scale=float(SAT))
                        rscr = dram2.tile([1, T], F16, tag="rscr")
                        dstr = bass.AP(tensor=rscr.tensor,
                                       offset=rscr.offset,
                                       ap=[[1, 128], [128, QTILES]])
                        nc.gpsimd.dma_start(dstr, rec16f[:])
                        rrec = atn.tile([128, T], F16, tag="rrec",
                                        name="rrec")
                        bcast = bass.AP(tensor=rscr.tensor,
                                        offset=rscr.offset,
                                        ap=[[0, 128], [1, T]])
                        nc.gpsimd.dma_start(rrec[:], bcast)
                        nc.vector.tensor_mul(praw[:], praw[:], rrec[:])
                        nc.scalar.copy(attnT8[:, h, 0, :], praw[:])
                        nc.vector.tensor_sub(attnT8[:, h, 1, :], praw[:],
                                             attnT8[:, h, 0, :])

                    # heads 2,3: group-wise, o_proj interleaved so the
                    # output writes overlap the attention tail
                    assemble_pair(1)
                    praw2 = apr.tile([128, T], F16, tag="praw", name="praw")
                    praw3 = apr.tile([128, T], F16, tag="praw", name="praw")
                    for g in range(4):
                        rs2 = attn_group(2, g, praw2)
                        norm_group(2, g, praw2, rs2)
                        if g > 0:
                            for m in range(4 * (g - 1), 4 * (g - 1) + 2):
                                oproj_m(m)
                        rs3 = attn_group(3, g, praw3)
                        norm_group(3, g, praw3, rs3)
                        if g > 0:
                            for m in range(4 * (g - 1) + 2, 4 * g):
                                oproj_m(m)
                    for m in range(12, 16):
                        oproj_m(m)

    nc.compile()
    _PROGRAM_CACHE[key] = nc
    return nc


def _host_prep(inputs):
    pos = np.asarray(inputs["positions"]).astype(np.float32)
    inv_freq = 1.0 / (THETA ** (np.arange(0, DR, 2, dtype=np.float32) / DR))
    freqs = pos[:, None] * inv_freq[None, :]
    cos, sin = np.cos(freqs), np.sin(freqs)

    eo = np.concatenate([np.arange(0, DR, 2), np.arange(1, DR, 2)])
    w_qkv_a = np.asarray(inputs["w_qkv_a"], np.float32)
    wa_cols = np.concatenate([
        w_qkv_a[:, QL:QL + KL],
        w_qkv_a[:, :QL],
    ], axis=1)
    wape_cols = w_qkv_a[:, QL + KL:][:, eo]
    w_q_b = np.asarray(inputs["w_q_b"], np.float32) * np.asarray(
        inputs["q_a_ln_w"], np.float32)[:, None]
    w_kv_b = np.asarray(inputs["w_kv_b"], np.float32) * np.asarray(
        inputs["kv_a_ln_w"], np.float32)[:, None]
    w_o = np.asarray(inputs["w_o"], np.float32)
    hid = np.asarray(inputs["hidden_states"], np.float32)
    hidT = np.ascontiguousarray(hid.T)

    scales = {
        "sh": _pow2(96.0, np.abs(hid).max()),
        "swa": _pow2(96.0, max(np.abs(wa_cols).max(),
                               np.abs(wape_cols).max())),
        "swqn": 1.0, "swqp": 1.0,
        "swkvb": _pow2(96.0, np.abs(w_kv_b).max()),
        "swo": _pow2(96.0, np.abs(w_o).max()),
    }

    # q_b column permutations (as v2), then scale + hi/lo split
    nope = w_q_b.reshape(QL, H, 192)[:, :, :DN]
    peh = w_q_b.reshape(QL, H, 192)[:, :, DN:]
    wqbn_cols = np.zeros((QL, H * DN), np.float32)
    wqbp_cols = np.zeros((QL, H * DR), np.float32)
    for d in range(NCORES):
        for hl in range(HPC):
            head = d * HPC + hl
            pair, e = hl // 2, hl % 2
            c0 = pair * 2048 + d * 256 + e * 128
            wqbn_cols[:, c0:c0 + 128] = nope[:, head, :]
            pE = peh[:, head, eo[:32]]
            pO = peh[:, head, eo[32:]]
            e0 = pair * 1024 + d * 64 + e * 32
            wqbp_cols[:, e0:e0 + 32] = pE
            wqbp_cols[:, 512 + e0:512 + e0 + 32] = pO
    scales["swqn"] = _pow2(96.0, np.abs(wqbn_cols).max())
    scales["swqp"] = _pow2(96.0, np.abs(wqbp_cols).max())

    def wsplit(wcols, s, kt):
        # [K, N] -> [128, kt, 2(lo,hi), N]
        hi, lo = _hilo(wcols * s)
        out = np.zeros((128, kt, 2, wcols.shape[1]), E4)
        for j in range(kt):
            out[:, j, 0, :] = lo[j * 128:(j + 1) * 128, :]
            out[:, j, 1, :] = hi[j * 128:(j + 1) * 128, :]
        return out

    wa8 = wsplit(wa_cols, scales["swa"], 40)
    wape8 = wsplit(wape_cols, scales["swa"], 40)
    wqbn8 = wsplit(wqbn_cols, scales["swqn"], 12)
    # wqbp8: chunk-contiguous [128, 12, 8(chunk), 2(lo,hi), 256]
    bp = wsplit(wqbp_cols, scales["swqp"], 12)      # [128, 12, 2, 2048]
    wqbp8 = np.ascontiguousarray(
        bp.reshape(128, 12, 2, 8, 256).transpose(0, 1, 3, 2, 4))

    c1 = scales["sh"] * scales["swa"]
    cosr = np.tile(cos, (1, 16)) * (SA / (SQA * scales["swqp"]))
    sinr = np.tile(sin, (1, 16)) * (SA / (SQA * scales["swqp"]))
    ctok = cos * (SA / c1)
    stok = sin * (SA / c1)
    triT = np.tril(np.full((128, 128), NEG, np.float32), -1)

    in_maps = []
    for c in range(NCORES):
        hs = [HPC * c + i for i in range(HPC)]
        kcols = np.concatenate(
            [w_kv_b[:, h * 256:h * 256 + DN] for h in hs], axis=1)
        vcols = np.concatenate(
            [w_kv_b[:, h * 256 + DN:(h + 1) * 256] for h in hs], axis=1)
        wkvb8 = wsplit(np.concatenate([kcols, vcols], axis=1),
                       scales["swkvb"], 4)
        # wo: [p(dv dim), head, 2(lo,hi), HID]
        wo_hi, wo_lo = _hilo(np.stack(
            [w_o[h * DV:(h + 1) * DV, :] for h in hs], axis=1)
            * scales["swo"])
        wo8 = np.stack([wo_lo, wo_hi], axis=2)

        sl = slice(c * TPC, (c + 1) * TPC)
        h_hi, h_lo = _hilo(np.ascontiguousarray(hidT[:, sl]) * scales["sh"])
        hT8 = np.zeros((128, 40, 2, TPC), E4)
        for j in range(40):
            hT8[:, j, 0, :] = h_hi[j * 128:(j + 1) * 128, :]
            hT8[:, j, 1, :] = h_lo[j * 128:(j + 1) * 128, :]

        in_maps.append({
            "hT8": hT8,
            "wa8": wa8,
            "wape8": wape8,
            "wqbn8": wqbn8,
            "wqbp8": wqbp8,
            "wkvb8": wkvb8,
            "wo8": np.ascontiguousarray(wo8),
            "ctok": np.ascontiguousarray(ctok[sl]).astype(np.float16),
            "stok": np.ascontiguousarray(stok[sl]).astype(np.float16),
            "cosr": np.ascontiguousarray(cosr[sl]).astype(np.float16),
            "sinr": np.ascontiguousarray(sinr[sl]).astype(np.float16),
            "triT": triT,
        })
    return in_maps, scales


def kernel(**inputs) -> np.ndarray:
    in_maps, scales = _host_prep(inputs)
    nc = build_program(scales)
    res = run_bass_kernel_spmd(nc, in_maps, core_ids=list(range(NCORES)))
    out = np.zeros((T, HID), np.float32)
    for r in res.results:
        out += r["out"].astype(np.float32)
    return out


if __name__ == "__main__":
    ins, scales = _host_prep({
        "positions": np.arange(T),
        "hidden_states": np.random.randn(T, HID).astype(np.float32),
        "w_qkv_a": np.random.randn(HID, CW).astype(np.float32) * HID ** -0.5,
        "q_a_ln_w": np.ones(QL, np.float32),
        "w_q_b": np.random.randn(QL, H * 192).astype(np.float32) * QL ** -0.5,
        "kv_a_ln_w": np.ones(KL, np.float32),
        "w_kv_b": np.random.randn(KL, H * 256).astype(np.float32) * KL ** -0.5,
        "w_o": np.random.randn(H * DV, HID).astype(np.float32) * (H * DV) ** -0.5,
    })
    build_program(scales)
    print("program built ok")
